# revision 2
# baseline (speedup 1.0000x reference)
# Trainium2 Bass kernel for nn_FCM_series_1 (gnn_message_passing).
#
# Math (derived from the reference):
#   aggregate(X, WW)[l,b,j] = tanh(-sum_i X[l,b,i] * WW[i,j])
#   T_A  = aggregate(A, WW)                     (12 lags x B rows)
#   U[t] = aggregate(train_init[:,:,t,1], WW)   (13 unique rows per batch;
#          A_N_OLD[la] = U[la], A_0_NEW[la] = U[la+1])
#   out[b,la,j] = P[la,j]*T_A[la,b,j] + Q[la,j]*U[la+1,b,j] + R[la,j]*U[la,b,j]
# with host-computable coefficients
#   P[la,j] = 2 * lambd[la, j%200] / belta[la] * 3**fract[la]
#   Q[la,j] = 3 * lambd[la, j%200] * l[la, j%200] / belta[la]
#   R[la,j] = Q[la,j] * Gamma(a+1)/(6*Gamma(a-2))
#   belta[la] = sum_{k=0..3} Gamma(a+1)/(Gamma(k+1)*Gamma(a-k+1))
#
# Sharding over 8 cores: batch split x2 (16 each), output node dim j split x4
# (300 each). Per core one matmul chain: lhsT=W-chunk tiles, rhs=X^T tiles,
# PSUM-accumulated over 10 k-tiles of 120 in bf16.
#
# All streamed operands are int8 in HBM (halves DMA bytes vs int16):
#   Wq[:,j] = round(-W[:,j]/swv_j), swv_j = max_i|W[i,j]|/127 (per-col scale)
#   XAq = round(A/sA) with A clipped at +-4.0 (sA = 4/127; clipping the
#         >4 sigma tail buys a ~25% finer step, net error win)
#   XUq = round(U/sU), sU = 1/127
# On-chip they are converted to bf16 *integer* values (exact in bf16, so the
# PE products are exact and HW matches the host numpy simulation, ~1.35e-2
# rel err vs the 2e-2 gate). X's U-block is scaled by sU/sA=0.25 (exact in
# bf16) during the convert so one column-uniform scale remains, and the whole
# dequant scale swv_j*sA is folded into tanh's per-partition scale vector:
#   t = tanh(psum * scl[j]),  scl[j] = swv_j * sA
# Epilogue: per-jt tanh on ACT (psum -> bf16), then a merged 5-op bf16
# combine on DVE using 3D APs spanning all 3 j-subtiles at once, and a single
# flat output DMA ([JS, NJ*CA] bf16, host untangles the layout).

import math

import numpy as np

LAG = 13
B = 32
N = 1200
H = 1.0 / 3.0

PB = 2          # batch shards
PJ = 4          # j shards
BL = B // PB    # 16 batches per core
JL = N // PJ    # 300 output nodes per core
NL = LAG - 1    # 12
CA = NL * BL    # 192 cols: T_A block, col = la*BL + b
CU = LAG * BL   # 208 cols: U block,  col = CA + t*BL + b
C = CA + CU     # 400 matmul moving cols
KT = 120        # contraction tile
NK = N // KT    # 10
JS = 100        # j subtile (psum partition dim)
NJ = JL // JS   # 3 j subtiles per core
SA = 4.0 / 127.0      # X A-block quant step (A clipped at 4.0)
SU = 1.0 / 127.0      # X U-block quant step (train_init in [0,1))
RU = SU / SA          # 0.25, exact in bf16
N_WARMUP = 8

_cached = None


def _gamma(x):
    return math.gamma(x)


def _build_nc():
    import concourse.bacc as bacc
    import concourse.mybir as mybir
    from concourse.tile import TileContext

    f32 = mybir.dt.float32
    bf16 = mybir.dt.bfloat16
    i8 = mybir.dt.int8
    nc = bacc.Bacc(None, target_bir_lowering=False)

    # partition-major repacked inputs (see kernel() for layouts)
    xt = nc.dram_tensor("xt", [KT, NK * C], i8, kind="ExternalInput")
    wc = nc.dram_tensor("wc", [KT, NK * JL], i8, kind="ExternalInput")
    coef = nc.dram_tensor("coef", [JS, 3 * NJ * NL], bf16, kind="ExternalInput")
    scl = nc.dram_tensor("scl", [JS, NJ], f32, kind="ExternalInput")
    out = nc.dram_tensor("out", [JS, NJ * CA], bf16, kind="ExternalOutput")

    with TileContext(nc) as tc:
        with (
            tc.tile_pool(name="sb", bufs=1) as pool,
            tc.tile_pool(name="ps", bufs=1, space="PSUM") as pspool,
        ):
            # ---- input DMA chunks, growing sizes so early k-tiles land
            # (and convert) before the PE stream needs them. W head is
            # k-major cols k*JL+j (k<8); X is k-major cols k*C+c.
            # sync:   Wk0 | Wk1-2 | Wk3-5 | Wk6-7 | Xk7 | Xk9
            # scalar: Xk0 | Xk1-2 | Xk3-5 | Xk6   | Xk8
            # gpsimd: coef | scl | W tails (per-jt k8-9)
            wg = {}   # k-range -> raw int8 W chunk tile
            xg = {}   # k-range -> raw int8 X chunk tile

            def loadw(eng, k0, nk):
                g = pool.tile([KT, nk * JL], i8, tag=f"wg{k0}", name=f"wg{k0}")
                eng.dma_start(out=g[:], in_=wc[:, k0 * JL:(k0 + nk) * JL])
                wg[(k0, nk)] = g

            def loadx(eng, k0, nk):
                g = pool.tile([KT, nk * C], i8, tag=f"xg{k0}", name=f"xg{k0}")
                eng.dma_start(out=g[:], in_=xt[:, k0 * C:(k0 + nk) * C])
                xg[(k0, nk)] = g

            loadw(nc.sync, 0, 1)
            loadx(nc.scalar, 0, 1)
            loadw(nc.sync, 1, 2)
            loadx(nc.scalar, 1, 2)
            loadw(nc.sync, 3, 3)
            loadx(nc.scalar, 3, 3)
            loadw(nc.sync, 6, 2)
            loadx(nc.scalar, 6, 1)
            loadx(nc.sync, 7, 1)
            loadx(nc.scalar, 8, 1)
            loadx(nc.sync, 9, 1)

            wtail = []
            for jt in range(NJ):
                g = pool.tile([KT, 2 * JS], i8, tag=f"wt{jt}", name=f"wt{jt}")
                c0 = 8 * JL + jt * 2 * JS
                nc.gpsimd.dma_start(out=g[:], in_=wc[:, c0:c0 + 2 * JS])
                wtail.append(g)

            coef_all = pool.tile([JS, 3 * NJ * NL], bf16, tag="coef")
            nc.gpsimd.dma_start(out=coef_all[:], in_=coef[:, :])
            scl_t = pool.tile([JS, NJ], f32, tag="scl")
            nc.gpsimd.dma_start(out=scl_t[:], in_=scl[:, :])

            # ---- converts: int8 -> bf16 integer values.
            # W on ACT (Copy), X on DVE (tensor_scalar_mul; U block x0.25).
            wf = pool.tile([KT, 8 * JL], bf16, tag="wf")
            for (k0, nk), g in wg.items():
                nc.scalar.activation(
                    out=wf[:, k0 * JL:(k0 + nk) * JL], in_=g[:],
                    func=mybir.ActivationFunctionType.Copy, scale=1.0)
            wtf = []
            for jt in range(NJ):
                t = pool.tile([KT, 2 * JS], bf16, tag=f"wtf{jt}",
                              name=f"wtf{jt}")
                nc.scalar.activation(
                    out=t[:], in_=wtail[jt][:],
                    func=mybir.ActivationFunctionType.Copy, scale=1.0)
                wtf.append(t)

            xf = pool.tile([KT, NK * C], bf16, tag="xf")
            xf3 = xf[:].rearrange("p (k c) -> p k c", k=NK)
            # convert ranges: k0 | k1-2 | k3-5 | k6-9 (last one waits for the
            # 4 small tail chunks; PE doesn't need k6 until much later)
            for (k0, nk), srcs in [
                ((0, 1), [xg[(0, 1)]]),
                ((1, 2), [xg[(1, 2)]]),
                ((3, 3), [xg[(3, 3)]]),
                ((6, 4), [xg[(6, 1)], xg[(7, 1)], xg[(8, 1)], xg[(9, 1)]]),
            ]:
                if len(srcs) == 1:
                    s3 = srcs[0][:].rearrange("p (k c) -> p k c", k=nk)
                    nc.vector.tensor_scalar_mul(
                        xf3[:, k0:k0 + nk, 0:CA], s3[:, :, 0:CA], 1.0)
                    nc.vector.tensor_scalar_mul(
                        xf3[:, k0:k0 + nk, CA:C], s3[:, :, CA:C], RU)
                else:
                    for i, s in enumerate(srcs):
                        s3 = s[:].rearrange("p (k c) -> p k c", k=1)
                        nc.vector.tensor_scalar_mul(
                            xf3[:, k0 + i:k0 + i + 1, 0:CA], s3[:, :, 0:CA],
                            1.0)
                        nc.vector.tensor_scalar_mul(
                            xf3[:, k0 + i:k0 + i + 1, CA:C], s3[:, :, CA:C],
                            RU)

            def w_slice(jt, k):
                if k >= 8:
                    return wtf[jt][:, (k - 8) * JS:(k - 7) * JS]
                return wf[:, k * JL + jt * JS:k * JL + jt * JS + JS]

            def x_slice(k):
                return xf[:, k * C:(k + 1) * C]

            # ---- replicate [JS,12] coefficient vectors to [JS,192] in one
            # 4D-AP copy during the DMA phase so combines run on flat APs.
            crep = pool.tile([JS, 3 * NJ * CA], bf16, tag="crep")
            src = coef_all[:].rearrange("p (g l) -> p g l", g=3 * NJ)
            dst = crep[:].rearrange("p (g l b) -> p g l b", g=3 * NJ, l=NL)
            nc.vector.tensor_copy(dst, src.broadcast_to([JS, 3 * NJ, NL, BL]))

            # ---- warm up the PE clock gate (HAM) with throwaway bf16
            # matmuls while inputs stream: otherwise the matmuls left after
            # the last DMA chunk run at the cold cadence.
            scratch = pool.tile([KT, C], bf16, tag="scr")
            nc.vector.memset(scratch[:], 0)
            psw = pspool.tile([JS, C], f32, tag="psw", name="psw")
            for _ in range(N_WARMUP):
                nc.tensor.matmul(psw[:], scratch[:, 0:JS], scratch[:],
                                 start=True, stop=True)

            ps = [pspool.tile([JS, C], f32, tag=f"ps{jt}", name=f"ps{jt}")
                  for jt in range(NJ)]
            # k0-7 k-major, then per-jt (k8, k9) so jt0's tanh starts
            # while jt1/jt2 still stream through the PE.
            mm_order = [(jt, k) for k in range(8) for jt in range(NJ)]
            mm_order += [(jt, k) for jt in range(NJ) for k in (8, 9)]
            for jt, k in mm_order:
                nc.tensor.matmul(
                    ps[jt][:], w_slice(jt, k), x_slice(k),
                    start=(k == 0), stop=(k == NK - 1),
                )

            # ---- epilogue: per-jt tanh on ACT (scale = per-partition
            # swv_j*sA vector), merged 3-jt combines on DVE, one output DMA.
            t_all = pool.tile([JS, NJ * C], bf16, tag="t")
            res = pool.tile([JS, NJ * CA], bf16, tag="res")
            tmp = pool.tile([JS, NJ * CA], bf16, tag="tmp")
            tmp2 = pool.tile([JS, NJ * CA], bf16, tag="tmp2")
            for jt in range(NJ):
                nc.scalar.activation(
                    out=t_all[:, jt * C:(jt + 1) * C], in_=ps[jt][:],
                    func=mybir.ActivationFunctionType.Tanh,
                    scale=scl_t[:, jt:jt + 1],
                )
            t3 = t_all[:].rearrange("p (j c) -> p j c", j=NJ)

            def cre(i):
                return crep[:, i * NJ * CA:(i + 1) * NJ * CA].rearrange(
                    "p (j c) -> p j c", j=NJ)

            res3 = res[:].rearrange("p (j c) -> p j c", j=NJ)
            tmp3 = tmp[:].rearrange("p (j c) -> p j c", j=NJ)
            tmp23 = tmp2[:].rearrange("p (j c) -> p j c", j=NJ)
            ve = nc.vector
            ve.tensor_mul(res3, cre(0), t3[:, :, 0:CA])
            ve.tensor_mul(tmp3, cre(1), t3[:, :, CA + BL:CA + CU])
            ve.tensor_mul(tmp23, cre(2), t3[:, :, CA:CA + CA])
            ve.tensor_add(res[:], res[:], tmp[:])
            ve.tensor_add(res[:], res[:], tmp2[:])
            nc.sync.dma_start(out=out[:, :], in_=res[:])

    return nc


def _get_nc():
    global _cached
    if _cached is None:
        _cached = _build_nc()
        _cached.finalize()   # Bacc: runs reg alloc + codegen passes
    return _cached


def _host_coefs(alpha, fract, lambd, l):
    # All [12,...] fp32; compute in float64, cast at the end.
    a = alpha[:, 0].astype(np.float64)          # [12]
    f = fract[:, 0].astype(np.float64)          # [12]
    lam = lambd[:, 0, :, 0].astype(np.float64)  # [12, 200]
    ll = l[:, 0, :, 0].astype(np.float64)       # [12, 200]

    belta = np.zeros(NL)
    for la in range(NL):
        g_a1 = _gamma(a[la] + 1.0)
        belta[la] = sum(
            g_a1 / (_gamma(kk + 1.0) * _gamma(a[la] - kk + 1.0)) for kk in range(4)
        )
    cN = np.array([_gamma(a[la] + 1.0) / (6.0 * _gamma(a[la] - 2.0))
                   for la in range(NL)])

    # tile lambda/l from 200 -> 1200 (index n % 200)
    lam_t = np.tile(lam, (1, 6))                # [12, 1200]
    ll_t = np.tile(ll, (1, 6))                  # [12, 1200]

    inv_hf = (1.0 / H) ** f                     # 3**fract
    P = 2.0 * lam_t / belta[:, None] * inv_hf[:, None]
    Q = lam_t * ll_t / belta[:, None] / H
    R = Q * cN[:, None]
    return P, Q, R


def kernel(A, WW, train_init, alpha, fract, lambd, l, A_y_list):
    import ml_dtypes
    from concourse.bass_utils import run_bass_kernel_spmd

    bf16 = ml_dtypes.bfloat16

    A = np.asarray(A, dtype=np.float32)
    WW = np.asarray(WW, dtype=np.float32)
    train_init = np.asarray(train_init, dtype=np.float32)

    P, Q, R = _host_coefs(
        np.asarray(alpha, np.float32), np.asarray(fract, np.float32),
        np.asarray(lambd, np.float32), np.asarray(l, np.float32))

    Wneg = -WW[:, :, 0]                         # [1200, 1200]
    swv = np.abs(Wneg).max(axis=0) / 127.0      # per-output-col scale [N]
    Wq = np.clip(np.round(Wneg / swv[None, :]), -127, 127).astype(np.int8)

    xts, wcs, coefs, scls = {}, {}, {}, {}
    for beta in range(PB):
        bsl = slice(beta * BL, (beta + 1) * BL)
        xa = np.clip(np.round(A[:, bsl, :, 0] / SA), -127, 127).astype(
            np.int8).transpose(2, 0, 1).reshape(N, CA)          # col=la*BL+b
        xu = np.clip(np.round(train_init[bsl, :, :, 1] / SU), -127,
                     127).astype(np.int8).transpose(1, 2, 0).reshape(N, CU)
        XT = np.concatenate([xa, xu], axis=1)                   # [1200, 400]
        # partition-major: [KT, NK*C], col = k*C + c
        xts[beta] = np.ascontiguousarray(
            XT.reshape(NK, KT, C).transpose(1, 0, 2).reshape(KT, NK * C))
    for g in range(PJ):
        gsl = slice(g * JL, (g + 1) * JL)
        # partition-major, k-major for k0-7, then per-jt (k8,k9) tails:
        # cols [k*JL + j for k<8] ++ [8*JL + jt*2*JS + (k-8)*JS + s]
        W3 = Wq[:, gsl].reshape(NK, KT, JL)
        head = W3[:8].transpose(1, 0, 2).reshape(KT, 8 * JL)
        tails = [W3[k][:, jt * JS:(jt + 1) * JS]
                 for jt in range(NJ) for k in (8, 9)]
        wcs[g] = np.ascontiguousarray(np.concatenate([head] + tails, axis=1))
        # coef [JS, 108]: col = kind*36 + jt*12 + la
        kinds = [M[:, gsl].reshape(NL, NJ, JS).transpose(2, 1, 0)
                 for M in (P, Q, R)]                            # [100, 3, 12]
        coefs[g] = np.ascontiguousarray(
            np.stack(kinds, axis=1).reshape(JS, 3 * NJ * NL).astype(bf16))
        # tanh scale: scl[p, jt] = swv[g*JL + jt*JS + p] * SA
        scls[g] = np.ascontiguousarray(
            (swv[gsl].reshape(NJ, JS).T * SA).astype(np.float32))

    in_maps = []
    for core in range(PB * PJ):
        beta, g = divmod(core, PJ)
        in_maps.append({"xt": xts[beta], "wc": wcs[g], "coef": coefs[g],
                        "scl": scls[g]})

    nc = _get_nc()
    res = run_bass_kernel_spmd(nc, in_maps, core_ids=list(range(PB * PJ)))
    kernel.last_results = res

    full = np.empty((B, NL, N), dtype=np.float32)
    for core in range(PB * PJ):
        beta, g = divmod(core, PJ)
        o = res.results[core]["out"]            # [JS, NJ*CA] bf16
        o = np.asarray(o).astype(np.float32).reshape(JS, NJ, NL, BL)
        # out[jt*JS+p, la, b] -> full[b, la, g*JL + jt*JS + p]
        full[beta * BL:(beta + 1) * BL, :, g * JL:(g + 1) * JL] = (
            o.transpose(3, 2, 1, 0).reshape(BL, NL, JL))
    return full.reshape(B, NL, N, 1)


# revision 4
# speedup vs baseline: 1.0038x; 1.0038x over previous
# Trainium2 Bass kernel for nn_FCM_series_1 (gnn_message_passing).
#
# Math (derived from the reference):
#   aggregate(X, WW)[l,b,j] = tanh(-sum_i X[l,b,i] * WW[i,j])
#   T_A  = aggregate(A, WW)                     (12 lags x B rows)
#   U[t] = aggregate(train_init[:,:,t,1], WW)   (13 unique rows per batch;
#          A_N_OLD[la] = U[la], A_0_NEW[la] = U[la+1])
#   out[b,la,j] = P[la,j]*T_A[la,b,j] + Q[la,j]*U[la+1,b,j] + R[la,j]*U[la,b,j]
# with host-computable coefficients
#   P[la,j] = 2 * lambd[la, j%200] / belta[la] * 3**fract[la]
#   Q[la,j] = 3 * lambd[la, j%200] * l[la, j%200] / belta[la]
#   R[la,j] = Q[la,j] * Gamma(a+1)/(6*Gamma(a-2))
#   belta[la] = sum_{k=0..3} Gamma(a+1)/(Gamma(k+1)*Gamma(a-k+1))
#
# Sharding over 8 cores: batch split x2 (16 each), output node dim j split x4
# (300 each). Per core one matmul chain: lhsT=W-chunk tiles, rhs=X^T tiles,
# PSUM-accumulated over 10 k-tiles of 120 in bf16.
#
# All streamed operands are int8 in HBM (halves DMA bytes vs int16):
#   Wq[:,j] = round(-W[:,j]/swv_j), swv_j = max_i|W[i,j]|/127 (per-col scale)
#   XAq = round(A/sA) with A clipped at +-4.0 (sA = 4/127; clipping the
#         >4 sigma tail buys a ~25% finer step, net error win)
#   XUq = round(U/sU), sU = 1/127
# On-chip they are converted to bf16 *integer* values (exact in bf16, so the
# PE products are exact and HW matches the host numpy simulation, ~1.35e-2
# rel err vs the 2e-2 gate). X's U-block is scaled by sU/sA=0.25 (exact in
# bf16) during the convert so one column-uniform scale remains, and the whole
# dequant scale swv_j*sA is folded into tanh's per-partition scale vector:
#   t = tanh(psum * scl[j]),  scl[j] = swv_j * sA
# Epilogue: per-jt tanh on ACT (psum -> bf16), then a merged 5-op bf16
# combine on DVE using 3D APs spanning all 3 j-subtiles at once, and a single
# flat output DMA ([JS, NJ*CA] bf16, host untangles the layout).

import math

import numpy as np

LAG = 13
B = 32
N = 1200
H = 1.0 / 3.0

PB = 2          # batch shards
PJ = 4          # j shards
BL = B // PB    # 16 batches per core
JL = N // PJ    # 300 output nodes per core
NL = LAG - 1    # 12
CA = NL * BL    # 192 cols: T_A block, col = la*BL + b
CU = LAG * BL   # 208 cols: U block,  col = CA + t*BL + b
C = CA + CU     # 400 matmul moving cols
KT = 120        # contraction tile
NK = N // KT    # 10
JS = 100        # j subtile (psum partition dim)
NJ = JL // JS   # 3 j subtiles per core
SA = 4.0 / 127.0      # X A-block quant step (A clipped at 4.0)
SU = 1.0 / 127.0      # X U-block quant step (train_init in [0,1))
RU = SU / SA          # 0.25, exact in bf16
N_WARMUP = 10

_cached = None


def _gamma(x):
    return math.gamma(x)


def _build_nc():
    import concourse.bacc as bacc
    import concourse.mybir as mybir
    from concourse.tile import TileContext

    f32 = mybir.dt.float32
    bf16 = mybir.dt.bfloat16
    i8 = mybir.dt.int8
    nc = bacc.Bacc(None, target_bir_lowering=False)

    # partition-major repacked inputs (see kernel() for layouts)
    xt = nc.dram_tensor("xt", [KT, NK * C], i8, kind="ExternalInput")
    wc = nc.dram_tensor("wc", [KT, NK * JL], i8, kind="ExternalInput")
    coef = nc.dram_tensor("coef", [JS, 3 * NJ * NL], bf16, kind="ExternalInput")
    scl = nc.dram_tensor("scl", [JS, NJ], f32, kind="ExternalInput")
    out = nc.dram_tensor("out", [JS, NJ * CA], bf16, kind="ExternalOutput")

    with TileContext(nc) as tc:
        with (
            tc.tile_pool(name="sb", bufs=1) as pool,
            tc.tile_pool(name="ps", bufs=1, space="PSUM") as pspool,
        ):
            # ---- input DMA chunks, growing sizes so early k-tiles land
            # (and convert) before the PE stream needs them. W head is
            # k-major cols k*JL+j (k<8); X is k-major cols k*C+c.
            #
            # Engine-time is the scarce resource in the first ~4us: each
            # dma_start costs ~850ns of issue time on its engine. The Scalar
            # engine must start W converts as soon as Wk0 lands (it gates the
            # whole PE stream), so it gets only the two LATE X issues,
            # interleaved after its early converts. GpSimd does the scratch
            # memset first so PE warmups start right after the preamble
            # (~6.1us) and ramp the PE clock (HAM) with zero idle gaps until
            # the real matmuls take over (cold cadence is ~2x warm; ramping
            # needs ~3-6us of CONTINUOUS PE activity).
            # sync:   Wk0 | Xk0 | Wk1-2 | Xk1-2 | Wk3-5 | Wk6-7 | out
            # scalar: (tanh table) Xk3-5 | convW0 | Xk6-9 | converts | tanhs
            # gpsimd: memset | coef | scl | W tails (per-jt k8-9)
            wg = {}   # k-range -> raw int8 W chunk tile
            xg = {}   # k-range -> raw int8 X chunk tile

            def loadw(eng, k0, nk):
                g = pool.tile([KT, nk * JL], i8, tag=f"wg{k0}", name=f"wg{k0}")
                eng.dma_start(out=g[:], in_=wc[:, k0 * JL:(k0 + nk) * JL])
                wg[(k0, nk)] = g

            def loadx(eng, k0, nk):
                g = pool.tile([KT, nk * C], i8, tag=f"xg{k0}", name=f"xg{k0}")
                eng.dma_start(out=g[:], in_=xt[:, k0 * C:(k0 + nk) * C])
                xg[(k0, nk)] = g

            # gpsimd: scratch memset first (unblocks PE warmups), then its
            # small/late DMAs on the SWDGE queue.
            scratch = pool.tile([KT, C], bf16, tag="scr")
            nc.gpsimd.memset(scratch[:], 0)
            coef_all = pool.tile([JS, 3 * NJ * NL], bf16, tag="coef")
            nc.gpsimd.dma_start(out=coef_all[:], in_=coef[:, :])
            scl_t = pool.tile([JS, NJ], f32, tag="scl")
            nc.gpsimd.dma_start(out=scl_t[:], in_=scl[:, :])
            wtail = []
            for jt in range(NJ):
                g = pool.tile([KT, 2 * JS], i8, tag=f"wt{jt}", name=f"wt{jt}")
                c0 = 8 * JL + jt * 2 * JS
                nc.gpsimd.dma_start(out=g[:], in_=wc[:, c0:c0 + 2 * JS])
                wtail.append(g)

            # sync: early W + early X, k-interleaved so both convert chains
            # start promptly.
            loadw(nc.sync, 0, 1)
            loadx(nc.sync, 0, 1)
            loadw(nc.sync, 1, 2)
            loadx(nc.sync, 1, 2)
            loadw(nc.sync, 3, 3)
            loadw(nc.sync, 6, 2)

            # PE warmups: start as soon as scratch is set, bridge seamlessly
            # into the real matmul stream.
            psw = pspool.tile([JS, C], f32, tag="psw", name="psw")
            for _ in range(N_WARMUP):
                nc.tensor.matmul(psw[:], scratch[:, 0:JS], scratch[:],
                                 start=True, stop=True)

            # scalar program: tanh table load is auto-emitted first (~1.3us),
            # then interleave its two late X issues with the W converts.
            wf = pool.tile([KT, 8 * JL], bf16, tag="wf")

            def convw(k0, nk):
                nc.scalar.activation(
                    out=wf[:, k0 * JL:(k0 + nk) * JL], in_=wg[(k0, nk)][:],
                    func=mybir.ActivationFunctionType.Copy, scale=1.0)

            loadx(nc.scalar, 3, 3)
            convw(0, 1)
            loadx(nc.scalar, 6, 4)
            convw(1, 2)
            convw(3, 3)
            convw(6, 2)
            wtf = []
            for jt in range(NJ):
                t = pool.tile([KT, 2 * JS], bf16, tag=f"wtf{jt}",
                              name=f"wtf{jt}")
                nc.scalar.activation(
                    out=t[:], in_=wtail[jt][:],
                    func=mybir.ActivationFunctionType.Copy, scale=1.0)
                wtf.append(t)

            # X converts on DVE (int8 -> bf16; U block x0.25 so one
            # column-uniform scale remains for tanh).
            xf = pool.tile([KT, NK * C], bf16, tag="xf")
            xf3 = xf[:].rearrange("p (k c) -> p k c", k=NK)

            def convx(k0, nk):
                s3 = xg[(k0, nk)][:].rearrange("p (k c) -> p k c", k=nk)
                nc.vector.tensor_scalar_mul(
                    xf3[:, k0:k0 + nk, 0:CA], s3[:, :, 0:CA], 1.0)
                nc.vector.tensor_scalar_mul(
                    xf3[:, k0:k0 + nk, CA:C], s3[:, :, CA:C], RU)

            convx(0, 1)
            convx(1, 2)
            # replicate [JS,12] coefficient vectors to [JS,192] in one 4D-AP
            # copy (coef lands early on the gpsimd queue) so the combines run
            # on flat APs.
            crep = pool.tile([JS, 3 * NJ * CA], bf16, tag="crep")
            src = coef_all[:].rearrange("p (g l) -> p g l", g=3 * NJ)
            dst = crep[:].rearrange("p (g l b) -> p g l b", g=3 * NJ, l=NL)
            nc.vector.tensor_copy(dst, src.broadcast_to([JS, 3 * NJ, NL, BL]))
            convx(3, 3)
            convx(6, 4)

            def w_slice(jt, k):
                if k >= 8:
                    return wtf[jt][:, (k - 8) * JS:(k - 7) * JS]
                return wf[:, k * JL + jt * JS:k * JL + jt * JS + JS]

            def x_slice(k):
                return xf[:, k * C:(k + 1) * C]

            ps = [pspool.tile([JS, C], f32, tag=f"ps{jt}", name=f"ps{jt}")
                  for jt in range(NJ)]
            # k0-7 k-major, then per-jt (k8, k9) so jt0's tanh starts
            # while jt1/jt2 still stream through the PE.
            mm_order = [(jt, k) for k in range(8) for jt in range(NJ)]
            mm_order += [(jt, k) for jt in range(NJ) for k in (8, 9)]
            for jt, k in mm_order:
                nc.tensor.matmul(
                    ps[jt][:], w_slice(jt, k), x_slice(k),
                    start=(k == 0), stop=(k == NK - 1),
                )

            # ---- epilogue: per-jt tanh on ACT (scale = per-partition
            # swv_j*sA vector), merged 3-jt combines on DVE, one output DMA.
            t_all = pool.tile([JS, NJ * C], bf16, tag="t")
            res = pool.tile([JS, NJ * CA], bf16, tag="res")
            tmp = pool.tile([JS, NJ * CA], bf16, tag="tmp")
            tmp2 = pool.tile([JS, NJ * CA], bf16, tag="tmp2")
            for jt in range(NJ):
                nc.scalar.activation(
                    out=t_all[:, jt * C:(jt + 1) * C], in_=ps[jt][:],
                    func=mybir.ActivationFunctionType.Tanh,
                    scale=scl_t[:, jt:jt + 1],
                )
            t3 = t_all[:].rearrange("p (j c) -> p j c", j=NJ)

            def cre(i):
                return crep[:, i * NJ * CA:(i + 1) * NJ * CA].rearrange(
                    "p (j c) -> p j c", j=NJ)

            res3 = res[:].rearrange("p (j c) -> p j c", j=NJ)
            tmp3 = tmp[:].rearrange("p (j c) -> p j c", j=NJ)
            tmp23 = tmp2[:].rearrange("p (j c) -> p j c", j=NJ)
            ve = nc.vector
            ve.tensor_mul(res3, cre(0), t3[:, :, 0:CA])
            ve.tensor_mul(tmp3, cre(1), t3[:, :, CA + BL:CA + CU])
            ve.tensor_mul(tmp23, cre(2), t3[:, :, CA:CA + CA])
            ve.tensor_add(res[:], res[:], tmp[:])
            ve.tensor_add(res[:], res[:], tmp2[:])
            nc.sync.dma_start(out=out[:, :], in_=res[:])

    return nc


def _get_nc():
    global _cached
    if _cached is None:
        _cached = _build_nc()
        _cached.finalize()   # Bacc: runs reg alloc + codegen passes
    return _cached


def _host_coefs(alpha, fract, lambd, l):
    # All [12,...] fp32; compute in float64, cast at the end.
    a = alpha[:, 0].astype(np.float64)          # [12]
    f = fract[:, 0].astype(np.float64)          # [12]
    lam = lambd[:, 0, :, 0].astype(np.float64)  # [12, 200]
    ll = l[:, 0, :, 0].astype(np.float64)       # [12, 200]

    belta = np.zeros(NL)
    for la in range(NL):
        g_a1 = _gamma(a[la] + 1.0)
        belta[la] = sum(
            g_a1 / (_gamma(kk + 1.0) * _gamma(a[la] - kk + 1.0)) for kk in range(4)
        )
    cN = np.array([_gamma(a[la] + 1.0) / (6.0 * _gamma(a[la] - 2.0))
                   for la in range(NL)])

    # tile lambda/l from 200 -> 1200 (index n % 200)
    lam_t = np.tile(lam, (1, 6))                # [12, 1200]
    ll_t = np.tile(ll, (1, 6))                  # [12, 1200]

    inv_hf = (1.0 / H) ** f                     # 3**fract
    P = 2.0 * lam_t / belta[:, None] * inv_hf[:, None]
    Q = lam_t * ll_t / belta[:, None] / H
    R = Q * cN[:, None]
    return P, Q, R


def kernel(A, WW, train_init, alpha, fract, lambd, l, A_y_list):
    import ml_dtypes
    from concourse.bass_utils import run_bass_kernel_spmd

    bf16 = ml_dtypes.bfloat16

    A = np.asarray(A, dtype=np.float32)
    WW = np.asarray(WW, dtype=np.float32)
    train_init = np.asarray(train_init, dtype=np.float32)

    P, Q, R = _host_coefs(
        np.asarray(alpha, np.float32), np.asarray(fract, np.float32),
        np.asarray(lambd, np.float32), np.asarray(l, np.float32))

    Wneg = -WW[:, :, 0]                         # [1200, 1200]
    swv = np.abs(Wneg).max(axis=0) / 127.0      # per-output-col scale [N]
    Wq = np.clip(np.round(Wneg / swv[None, :]), -127, 127).astype(np.int8)

    xts, wcs, coefs, scls = {}, {}, {}, {}
    for beta in range(PB):
        bsl = slice(beta * BL, (beta + 1) * BL)
        xa = np.clip(np.round(A[:, bsl, :, 0] / SA), -127, 127).astype(
            np.int8).transpose(2, 0, 1).reshape(N, CA)          # col=la*BL+b
        xu = np.clip(np.round(train_init[bsl, :, :, 1] / SU), -127,
                     127).astype(np.int8).transpose(1, 2, 0).reshape(N, CU)
        XT = np.concatenate([xa, xu], axis=1)                   # [1200, 400]
        # partition-major: [KT, NK*C], col = k*C + c
        xts[beta] = np.ascontiguousarray(
            XT.reshape(NK, KT, C).transpose(1, 0, 2).reshape(KT, NK * C))
    for g in range(PJ):
        gsl = slice(g * JL, (g + 1) * JL)
        # partition-major, k-major for k0-7, then per-jt (k8,k9) tails:
        # cols [k*JL + j for k<8] ++ [8*JL + jt*2*JS + (k-8)*JS + s]
        W3 = Wq[:, gsl].reshape(NK, KT, JL)
        head = W3[:8].transpose(1, 0, 2).reshape(KT, 8 * JL)
        tails = [W3[k][:, jt * JS:(jt + 1) * JS]
                 for jt in range(NJ) for k in (8, 9)]
        wcs[g] = np.ascontiguousarray(np.concatenate([head] + tails, axis=1))
        # coef [JS, 108]: col = kind*36 + jt*12 + la
        kinds = [M[:, gsl].reshape(NL, NJ, JS).transpose(2, 1, 0)
                 for M in (P, Q, R)]                            # [100, 3, 12]
        coefs[g] = np.ascontiguousarray(
            np.stack(kinds, axis=1).reshape(JS, 3 * NJ * NL).astype(bf16))
        # tanh scale: scl[p, jt] = swv[g*JL + jt*JS + p] * SA
        scls[g] = np.ascontiguousarray(
            (swv[gsl].reshape(NJ, JS).T * SA).astype(np.float32))

    in_maps = []
    for core in range(PB * PJ):
        beta, g = divmod(core, PJ)
        in_maps.append({"xt": xts[beta], "wc": wcs[g], "coef": coefs[g],
                        "scl": scls[g]})

    nc = _get_nc()
    res = run_bass_kernel_spmd(nc, in_maps, core_ids=list(range(PB * PJ)))
    kernel.last_results = res

    full = np.empty((B, NL, N), dtype=np.float32)
    for core in range(PB * PJ):
        beta, g = divmod(core, PJ)
        o = res.results[core]["out"]            # [JS, NJ*CA] bf16
        o = np.asarray(o).astype(np.float32).reshape(JS, NJ, NL, BL)
        # out[jt*JS+p, la, b] -> full[b, la, g*JL + jt*JS + p]
        full[beta * BL:(beta + 1) * BL, :, g * JL:(g + 1) * JL] = (
            o.transpose(3, 2, 1, 0).reshape(BL, NL, JL))
    return full.reshape(B, NL, N, 1)


# revision 5
# speedup vs baseline: 1.0059x; 1.0021x over previous
# Trainium2 Bass kernel for nn_FCM_series_1 (gnn_message_passing).
#
# Math (derived from the reference):
#   aggregate(X, WW)[l,b,j] = tanh(-sum_i X[l,b,i] * WW[i,j])
#   T_A  = aggregate(A, WW)                     (12 lags x B rows)
#   U[t] = aggregate(train_init[:,:,t,1], WW)   (13 unique rows per batch;
#          A_N_OLD[la] = U[la], A_0_NEW[la] = U[la+1])
#   out[b,la,j] = P[la,j]*T_A[la,b,j] + Q[la,j]*U[la+1,b,j] + R[la,j]*U[la,b,j]
# with host-computable coefficients
#   P[la,j] = 2 * lambd[la, j%200] / belta[la] * 3**fract[la]
#   Q[la,j] = 3 * lambd[la, j%200] * l[la, j%200] / belta[la]
#   R[la,j] = Q[la,j] * Gamma(a+1)/(6*Gamma(a-2))
#   belta[la] = sum_{k=0..3} Gamma(a+1)/(Gamma(k+1)*Gamma(a-k+1))
#
# Sharding over 8 cores: batch split x2 (16 each), output node dim j split x4
# (300 each). Per core one matmul chain: lhsT=W k-tiles, rhs=X^T k-tiles,
# PSUM-accumulated over 10 k-tiles of 120, all operands bf16 (W negated on
# the host so psum = -X@W; bf16 streaming is ~2e-3 rel err, far under the
# 2e-2 gate).
#
# Schedule notes (why it looks like this):
# - Everything before the first real matmul is overhead; the PE clock (HAM)
#   additionally needs ~6us of CONTINUOUS busy to reach full speed (cold
#   cadence ~2x slower, and any idle gap resets the ramp). So the GpSimd
#   engine memsets the warmup scratch first thing and the PE runs throwaway
#   matmuls back-to-back from ~6.2us until the real stream takes over.
# - bf16-direct streaming (no int8+dequant) keeps the convert stage OFF the
#   critical path entirely: the first matmul only waits for the Wk0-1/Xk0-1
#   DMAs. ACT only does the 3 tanhs, DVE only the coefficient replicate and
#   the 5 merged combines.
# - Each dma_start costs ~7ns/descriptor-row of engine issue time and the
#   queue transfers follow descriptor writes, so few, large-row transfers
#   win. Rows here are 1200-3200B.
# - The Scalar engine's auto-emitted tanh-table load (~1.3us) gates its
#   first issue, so the k0-1 chunks ride the Sync queue.
# - Matmul order: k0-5 across all jt, then per-jt k6-9, so jt0's tanh and
#   the merged combines start as early as possible after the k6-9 chunks
#   land. Epilogue = 3 tanh (ACT, psum->bf16), 5 merged DVE ops via 3D APs
#   spanning all 3 j-subtiles, one flat output DMA ([JS, NJ*CA] bf16, host
#   untangles).

import math

import numpy as np

LAG = 13
B = 32
N = 1200
H = 1.0 / 3.0

PB = 2          # batch shards
PJ = 4          # j shards
BL = B // PB    # 16 batches per core
JL = N // PJ    # 300 output nodes per core
NL = LAG - 1    # 12
CA = NL * BL    # 192 cols: T_A block, col = la*BL + b
CU = LAG * BL   # 208 cols: U block,  col = CA + t*BL + b
C = CA + CU     # 400 matmul moving cols
KT = 120        # contraction tile
NK = N // KT    # 10
JS = 100        # j subtile (psum partition dim)
NJ = JL // JS   # 3 j subtiles per core
N_WARMUP = 9

_cached = None


def _gamma(x):
    return math.gamma(x)


def _build_nc():
    import concourse.bacc as bacc
    import concourse.mybir as mybir
    from concourse.tile import TileContext

    bf16 = mybir.dt.bfloat16
    nc = bacc.Bacc(None, target_bir_lowering=False)

    # partition-major repacked inputs (see kernel() for layouts)
    xt = nc.dram_tensor("xt", [KT, NK * C], bf16, kind="ExternalInput")
    wc = nc.dram_tensor("wc", [KT, NK * JL], bf16, kind="ExternalInput")
    coef = nc.dram_tensor("coef", [JS, 3 * NJ * NL], bf16, kind="ExternalInput")
    out = nc.dram_tensor("out", [JS, NJ * CA], bf16, kind="ExternalOutput")

    with TileContext(nc) as tc:
        with (
            tc.tile_pool(name="sb", bufs=1) as pool,
            tc.tile_pool(name="ps", bufs=1, space="PSUM") as pspool,
        ):
            # gpsimd: scratch memset first (unblocks PE warmups), then coef.
            scratch = pool.tile([KT, C], bf16, tag="scr")
            nc.gpsimd.memset(scratch[:], 0)
            coef_all = pool.tile([JS, 3 * NJ * NL], bf16, tag="coef")
            nc.gpsimd.dma_start(out=coef_all[:], in_=coef[:, :])

            # input chunks. sync: Wk0-1 | Xk0-1 | Xk2-5 | Xk6-9;
            # scalar (after its table load): Wk2-5 | Wk6-9.
            wf = pool.tile([KT, NK * JL], bf16, tag="wf")
            xf = pool.tile([KT, NK * C], bf16, tag="xf")

            def loadw(eng, k0, nk):
                eng.dma_start(out=wf[:, k0 * JL:(k0 + nk) * JL],
                              in_=wc[:, k0 * JL:(k0 + nk) * JL])

            def loadx(eng, k0, nk):
                eng.dma_start(out=xf[:, k0 * C:(k0 + nk) * C],
                              in_=xt[:, k0 * C:(k0 + nk) * C])

            loadw(nc.sync, 0, 2)
            loadx(nc.sync, 0, 2)
            loadx(nc.sync, 2, 4)
            loadx(nc.sync, 6, 4)
            loadw(nc.scalar, 2, 4)
            loadw(nc.scalar, 6, 4)

            # PE warmups: start as soon as scratch is set, bridge seamlessly
            # into the real matmul stream to keep the HAM ramp alive.
            psw = pspool.tile([JS, C], mybir.dt.float32, tag="psw", name="psw")
            for _ in range(N_WARMUP):
                nc.tensor.matmul(psw[:], scratch[:, 0:JS], scratch[:],
                                 start=True, stop=True)

            def w_slice(jt, k):
                return wf[:, k * JL + jt * JS:k * JL + jt * JS + JS]

            def x_slice(k):
                return xf[:, k * C:(k + 1) * C]

            # replicate [JS,12] coefficient vectors to [JS,192] in one 4D-AP
            # copy (coef lands early on the gpsimd queue) so the combines run
            # on flat APs.
            crep = pool.tile([JS, 3 * NJ * CA], bf16, tag="crep")
            src = coef_all[:].rearrange("p (g l) -> p g l", g=3 * NJ)
            dst = crep[:].rearrange("p (g l b) -> p g l b", g=3 * NJ, l=NL)
            nc.vector.tensor_copy(dst, src.broadcast_to([JS, 3 * NJ, NL, BL]))

            ps = [pspool.tile([JS, C], mybir.dt.float32, tag=f"ps{jt}",
                              name=f"ps{jt}")
                  for jt in range(NJ)]
            mm_order = [(jt, k) for k in range(6) for jt in range(NJ)]
            mm_order += [(jt, k) for jt in range(NJ) for k in range(6, NK)]
            for jt, k in mm_order:
                nc.tensor.matmul(
                    ps[jt][:], w_slice(jt, k), x_slice(k),
                    start=(k == 0), stop=(k == NK - 1),
                )

            # epilogue: per-jt tanh on ACT, merged 3-jt combines on DVE,
            # one flat output DMA.
            t_all = pool.tile([JS, NJ * C], bf16, tag="t")
            res = pool.tile([JS, NJ * CA], bf16, tag="res")
            tmp = pool.tile([JS, NJ * CA], bf16, tag="tmp")
            tmp2 = pool.tile([JS, NJ * CA], bf16, tag="tmp2")
            for jt in range(NJ):
                nc.scalar.activation(
                    out=t_all[:, jt * C:(jt + 1) * C], in_=ps[jt][:],
                    func=mybir.ActivationFunctionType.Tanh,
                )
            t3 = t_all[:].rearrange("p (j c) -> p j c", j=NJ)

            def cre(i):
                return crep[:, i * NJ * CA:(i + 1) * NJ * CA].rearrange(
                    "p (j c) -> p j c", j=NJ)

            res3 = res[:].rearrange("p (j c) -> p j c", j=NJ)
            tmp3 = tmp[:].rearrange("p (j c) -> p j c", j=NJ)
            tmp23 = tmp2[:].rearrange("p (j c) -> p j c", j=NJ)
            ve = nc.vector
            ve.tensor_mul(res3, cre(0), t3[:, :, 0:CA])
            ve.tensor_mul(tmp3, cre(1), t3[:, :, CA + BL:CA + CU])
            ve.tensor_mul(tmp23, cre(2), t3[:, :, CA:CA + CA])
            ve.tensor_add(res[:], res[:], tmp[:])
            ve.tensor_add(res[:], res[:], tmp2[:])
            nc.sync.dma_start(out=out[:, :], in_=res[:])

    return nc


def _get_nc():
    global _cached
    if _cached is None:
        _cached = _build_nc()
        _cached.finalize()   # Bacc: runs reg alloc + codegen passes
    return _cached


def _host_coefs(alpha, fract, lambd, l):
    # All [12,...] fp32; compute in float64, cast at the end.
    a = alpha[:, 0].astype(np.float64)          # [12]
    f = fract[:, 0].astype(np.float64)          # [12]
    lam = lambd[:, 0, :, 0].astype(np.float64)  # [12, 200]
    ll = l[:, 0, :, 0].astype(np.float64)       # [12, 200]

    belta = np.zeros(NL)
    for la in range(NL):
        g_a1 = _gamma(a[la] + 1.0)
        belta[la] = sum(
            g_a1 / (_gamma(kk + 1.0) * _gamma(a[la] - kk + 1.0)) for kk in range(4)
        )
    cN = np.array([_gamma(a[la] + 1.0) / (6.0 * _gamma(a[la] - 2.0))
                   for la in range(NL)])

    # tile lambda/l from 200 -> 1200 (index n % 200)
    lam_t = np.tile(lam, (1, 6))                # [12, 1200]
    ll_t = np.tile(ll, (1, 6))                  # [12, 1200]

    inv_hf = (1.0 / H) ** f                     # 3**fract
    P = 2.0 * lam_t / belta[:, None] * inv_hf[:, None]
    Q = lam_t * ll_t / belta[:, None] / H
    R = Q * cN[:, None]
    return P, Q, R


def kernel(A, WW, train_init, alpha, fract, lambd, l, A_y_list):
    import ml_dtypes
    from concourse.bass_utils import run_bass_kernel_spmd

    bf16 = ml_dtypes.bfloat16

    A = np.asarray(A, dtype=np.float32)
    WW = np.asarray(WW, dtype=np.float32)
    train_init = np.asarray(train_init, dtype=np.float32)

    P, Q, R = _host_coefs(
        np.asarray(alpha, np.float32), np.asarray(fract, np.float32),
        np.asarray(lambd, np.float32), np.asarray(l, np.float32))

    Wneg = (-WW[:, :, 0]).astype(bf16)          # [1200, 1200]

    xts, wcs, coefs = {}, {}, {}
    for beta in range(PB):
        bsl = slice(beta * BL, (beta + 1) * BL)
        xa = A[:, bsl, :, 0].astype(bf16).transpose(2, 0, 1).reshape(N, CA)
        xu = train_init[bsl, :, :, 1].astype(bf16).transpose(1, 2, 0).reshape(
            N, CU)
        XT = np.concatenate([xa, xu], axis=1)                   # [1200, 400]
        # partition-major: [KT, NK*C], col = k*C + c
        xts[beta] = np.ascontiguousarray(
            XT.reshape(NK, KT, C).transpose(1, 0, 2).reshape(KT, NK * C))
    for g in range(PJ):
        gsl = slice(g * JL, (g + 1) * JL)
        # partition-major, k-major: col = k*JL + j
        wcs[g] = np.ascontiguousarray(
            Wneg[:, gsl].reshape(NK, KT, JL).transpose(1, 0, 2).reshape(
                KT, NK * JL))
        # coef [JS, 108]: col = kind*36 + jt*12 + la
        kinds = [M[:, gsl].reshape(NL, NJ, JS).transpose(2, 1, 0)
                 for M in (P, Q, R)]                            # [100, 3, 12]
        coefs[g] = np.ascontiguousarray(
            np.stack(kinds, axis=1).reshape(JS, 3 * NJ * NL).astype(bf16))

    in_maps = []
    for core in range(PB * PJ):
        beta, g = divmod(core, PJ)
        in_maps.append({"xt": xts[beta], "wc": wcs[g], "coef": coefs[g]})

    nc = _get_nc()
    res = run_bass_kernel_spmd(nc, in_maps, core_ids=list(range(PB * PJ)))
    kernel.last_results = res

    full = np.empty((B, NL, N), dtype=np.float32)
    for core in range(PB * PJ):
        beta, g = divmod(core, PJ)
        o = res.results[core]["out"]            # [JS, NJ*CA] bf16
        o = np.asarray(o).astype(np.float32).reshape(JS, NJ, NL, BL)
        # out[p, jt, la, b] -> full[b, la, g*JL + jt*JS + p]
        full[beta * BL:(beta + 1) * BL, :, g * JL:(g + 1) * JL] = (
            o.transpose(3, 2, 1, 0).reshape(BL, NL, JL))
    return full.reshape(B, NL, N, 1)


# revision 9
# speedup vs baseline: 1.0393x; 1.0333x over previous
# Trainium2 Bass kernel for nn_FCM_series_1 (gnn_message_passing).
#
# Math (derived from the reference):
#   aggregate(X, WW)[l,b,j] = tanh(-sum_i X[l,b,i] * WW[i,j])
#   T_A  = aggregate(A, WW)                     (12 lags x B rows)
#   U[t] = aggregate(train_init[:,:,t,1], WW)   (13 unique rows per batch;
#          A_N_OLD[la] = U[la], A_0_NEW[la] = U[la+1])
#   out[b,la,j] = P[la,j]*T_A[la,b,j] + Q[la,j]*U[la+1,b,j] + R[la,j]*U[la,b,j]
# with host-computable coefficients
#   P[la,j] = 2 * lambd[la, j%200] / belta[la] * 3**fract[la]
#   Q[la,j] = 3 * lambd[la, j%200] * l[la, j%200] / belta[la]
#   R[la,j] = Q[la,j] * Gamma(a+1)/(6*Gamma(a-2))
#   belta[la] = sum_{k=0..3} Gamma(a+1)/(Gamma(k+1)*Gamma(a-k+1))
#
# Sharding over 8 cores: batch split x2 (16 each), output node dim j split x4
# (300 each). Per core one matmul chain: lhsT=W k-tiles, rhs=X^T k-tiles,
# PSUM-accumulated over 10 k-tiles of 120, all operands bf16 (W negated on
# the host so psum = -X@W; bf16 streaming is ~2e-3 rel err, far under the
# 2e-2 gate).
#
# Schedule notes (why it looks like this):
# - Everything before the first real matmul is overhead; the PE clock (HAM)
#   additionally needs ~6us of CONTINUOUS busy to reach full speed (cold
#   cadence ~2x slower, and any idle gap resets the ramp). So the GpSimd
#   engine memsets the warmup scratch first thing and the PE runs throwaway
#   matmuls back-to-back from ~6.2us until the real stream takes over.
# - bf16-direct streaming (no int8+dequant) keeps the convert stage OFF the
#   critical path entirely: the first matmul only waits for the Wk0-1/Xk0-1
#   DMAs. ACT only does the 3 tanhs, DVE only the coefficient replicate and
#   the 5 merged combines.
# - Each dma_start costs ~7ns/descriptor-row of engine issue time and the
#   queue transfers follow descriptor writes, so few, large-row transfers
#   win. Rows here are 1200-3200B.
# - The Scalar engine's auto-emitted tanh-table load (~1.3us) gates its
#   first issue, so the k0-1 chunks ride the Sync queue.
# - Matmul order: k0-5 across all jt, then per-jt k6-9, so jt0's tanh and
#   the merged combines start as early as possible after the k6-9 chunks
#   land. Epilogue = 3 tanh (ACT, psum->bf16), 5 merged DVE ops via 3D APs
#   spanning all 3 j-subtiles, one flat output DMA ([JS, NJ*CA] bf16, host
#   untangles).

import math

import numpy as np

LAG = 13
B = 32
N = 1200
H = 1.0 / 3.0

PB = 2          # batch shards
PJ = 4          # j shards
BL = B // PB    # 16 batches per core
JL = N // PJ    # 300 output nodes per core
NL = LAG - 1    # 12
CA = NL * BL    # 192 cols: T_A block, col = la*BL + b
CU = LAG * BL   # 208 cols: U block,  col = CA + t*BL + b
C = CA + CU     # 400 matmul moving cols
KT = 120        # contraction tile
NK = N // KT    # 10
JS = 100        # j subtile (psum partition dim)
NJ = JL // JS   # 3 j subtiles per core
N_WARMUP = 11

_cached = None


def _gamma(x):
    return math.gamma(x)


def _build_nc():
    import concourse.bacc as bacc
    import concourse.mybir as mybir
    from concourse.tile import TileContext

    bf16 = mybir.dt.bfloat16
    # enable_partition_id=False drops the per-engine partition-id TENSOR_LOAD
    # from the preamble (~1.2us on the critical path; this kernel never reads
    # the partition id).
    nc = bacc.Bacc(None, target_bir_lowering=False, enable_partition_id=False)

    # partition-major repacked inputs (see kernel() for layouts)
    xt = nc.dram_tensor("xt", [KT, NK * C], bf16, kind="ExternalInput")
    wc = nc.dram_tensor("wc", [KT, NK * JL], bf16, kind="ExternalInput")
    coef = nc.dram_tensor("coef", [JS, 3 * NJ * NL], bf16, kind="ExternalInput")
    out = nc.dram_tensor("out", [JS, NJ * CA], bf16, kind="ExternalOutput")

    with TileContext(nc) as tc:
        with (
            tc.tile_pool(name="sb", bufs=1) as pool,
            tc.tile_pool(name="ps", bufs=1, space="PSUM") as pspool,
        ):
            # gpsimd: scratch memset first (unblocks PE warmups), then coef.
            scratch = pool.tile([KT, C], bf16, tag="scr")
            nc.gpsimd.memset(scratch[:], 0)
            coef_all = pool.tile([JS, 3 * NJ * NL], bf16, tag="coef")
            nc.gpsimd.dma_start(out=coef_all[:], in_=coef[:, :])

            # Input chunks, all on the Sync HWDGE queue: HBM caps aggregate
            # DMA at ~340B/ns regardless of queue count, a second queue only
            # adds contention jitter, and keeping Scalar issue-free means its
            # tanh-table load overlaps the stream. Each chunk gets its OWN
            # tile: slicing one big tile lumps the DMA-write dependencies,
            # making the first matmul wait on later chunks.
            wg = {}   # k-range -> W chunk tile
            xg = {}   # k-range -> X chunk tile

            def loadw(k0, nk):
                g = pool.tile([KT, nk * JL], bf16, tag=f"wg{k0}",
                              name=f"wg{k0}")
                nc.sync.dma_start(out=g[:], in_=wc[:, k0 * JL:(k0 + nk) * JL])
                wg[k0] = g

            def loadx(k0, nk):
                g = pool.tile([KT, nk * C], bf16, tag=f"xg{k0}",
                              name=f"xg{k0}")
                nc.sync.dma_start(out=g[:], in_=xt[:, k0 * C:(k0 + nk) * C])
                xg[k0] = g

            loadw(0, 2)
            loadx(0, 2)
            loadx(2, 4)
            loadw(2, 4)
            loadw(6, 4)
            loadx(6, 4)
            wmap = {k: (wg[0], 0) if k < 2 else ((wg[2], 2) if k < 6 else
                    (wg[6], 6)) for k in range(NK)}
            xmap = {k: (xg[0], 0) if k < 2 else ((xg[2], 2) if k < 6 else
                    (xg[6], 6)) for k in range(NK)}

            # PE warmups: start as soon as scratch is set, bridge seamlessly
            # into the real matmul stream to keep the HAM ramp alive.
            psw = pspool.tile([JS, C], mybir.dt.float32, tag="psw", name="psw")
            for _ in range(N_WARMUP):
                nc.tensor.matmul(psw[:], scratch[:, 0:JS], scratch[:],
                                 start=True, stop=True)

            def w_slice(jt, k):
                g, k0 = wmap[k]
                c0 = (k - k0) * JL + jt * JS
                return g[:, c0:c0 + JS]

            def x_slice(k):
                g, k0 = xmap[k]
                return g[:, (k - k0) * C:(k - k0 + 1) * C]

            # replicate [JS,12] coefficient vectors to [JS,192] in one 4D-AP
            # copy (coef lands early on the gpsimd queue) so the combines run
            # on flat APs.
            crep = pool.tile([JS, 3 * NJ * CA], bf16, tag="crep")
            src = coef_all[:].rearrange("p (g l) -> p g l", g=3 * NJ)
            dst = crep[:].rearrange("p (g l b) -> p g l b", g=3 * NJ, l=NL)
            nc.vector.tensor_copy(dst, src.broadcast_to([JS, 3 * NJ, NL, BL]))

            ps = [pspool.tile([JS, C], mybir.dt.float32, tag=f"ps{jt}",
                              name=f"ps{jt}")
                  for jt in range(NJ)]
            mm_order = [(jt, k) for k in range(6) for jt in range(NJ)]
            mm_order += [(jt, k) for jt in range(NJ) for k in range(6, NK)]
            for jt, k in mm_order:
                nc.tensor.matmul(
                    ps[jt][:], w_slice(jt, k), x_slice(k),
                    start=(k == 0), stop=(k == NK - 1),
                )

            # epilogue: per-jt tanh on ACT, merged 3-jt combines on DVE,
            # one flat output DMA.
            t_all = pool.tile([JS, NJ * C], bf16, tag="t")
            res = pool.tile([JS, NJ * CA], bf16, tag="res")
            tmp = pool.tile([JS, NJ * CA], bf16, tag="tmp")
            tmp2 = pool.tile([JS, NJ * CA], bf16, tag="tmp2")
            for jt in range(NJ):
                nc.scalar.activation(
                    out=t_all[:, jt * C:(jt + 1) * C], in_=ps[jt][:],
                    func=mybir.ActivationFunctionType.Tanh,
                )
            t3 = t_all[:].rearrange("p (j c) -> p j c", j=NJ)

            def cre(i):
                return crep[:, i * NJ * CA:(i + 1) * NJ * CA].rearrange(
                    "p (j c) -> p j c", j=NJ)

            res3 = res[:].rearrange("p (j c) -> p j c", j=NJ)
            tmp3 = tmp[:].rearrange("p (j c) -> p j c", j=NJ)
            tmp23 = tmp2[:].rearrange("p (j c) -> p j c", j=NJ)
            ve = nc.vector
            ve.tensor_mul(res3, cre(0), t3[:, :, 0:CA])
            ve.tensor_mul(tmp3, cre(1), t3[:, :, CA + BL:CA + CU])
            ve.tensor_mul(tmp23, cre(2), t3[:, :, CA:CA + CA])
            ve.tensor_add(res[:], res[:], tmp[:])
            ve.tensor_add(res[:], res[:], tmp2[:])
            nc.sync.dma_start(out=out[:, :], in_=res[:])

    return nc


def _get_nc():
    global _cached
    if _cached is None:
        _cached = _build_nc()
        _cached.finalize()   # Bacc: runs reg alloc + codegen passes
    return _cached


def _host_coefs(alpha, fract, lambd, l):
    # All [12,...] fp32; compute in float64, cast at the end.
    a = alpha[:, 0].astype(np.float64)          # [12]
    f = fract[:, 0].astype(np.float64)          # [12]
    lam = lambd[:, 0, :, 0].astype(np.float64)  # [12, 200]
    ll = l[:, 0, :, 0].astype(np.float64)       # [12, 200]

    belta = np.zeros(NL)
    for la in range(NL):
        g_a1 = _gamma(a[la] + 1.0)
        belta[la] = sum(
            g_a1 / (_gamma(kk + 1.0) * _gamma(a[la] - kk + 1.0)) for kk in range(4)
        )
    cN = np.array([_gamma(a[la] + 1.0) / (6.0 * _gamma(a[la] - 2.0))
                   for la in range(NL)])

    # tile lambda/l from 200 -> 1200 (index n % 200)
    lam_t = np.tile(lam, (1, 6))                # [12, 1200]
    ll_t = np.tile(ll, (1, 6))                  # [12, 1200]

    inv_hf = (1.0 / H) ** f                     # 3**fract
    P = 2.0 * lam_t / belta[:, None] * inv_hf[:, None]
    Q = lam_t * ll_t / belta[:, None] / H
    R = Q * cN[:, None]
    return P, Q, R


def kernel(A, WW, train_init, alpha, fract, lambd, l, A_y_list):
    import ml_dtypes
    from concourse.bass_utils import run_bass_kernel_spmd

    bf16 = ml_dtypes.bfloat16

    A = np.asarray(A, dtype=np.float32)
    WW = np.asarray(WW, dtype=np.float32)
    train_init = np.asarray(train_init, dtype=np.float32)

    P, Q, R = _host_coefs(
        np.asarray(alpha, np.float32), np.asarray(fract, np.float32),
        np.asarray(lambd, np.float32), np.asarray(l, np.float32))

    Wneg = (-WW[:, :, 0]).astype(bf16)          # [1200, 1200]

    xts, wcs, coefs = {}, {}, {}
    for beta in range(PB):
        bsl = slice(beta * BL, (beta + 1) * BL)
        xa = A[:, bsl, :, 0].astype(bf16).transpose(2, 0, 1).reshape(N, CA)
        xu = train_init[bsl, :, :, 1].astype(bf16).transpose(1, 2, 0).reshape(
            N, CU)
        XT = np.concatenate([xa, xu], axis=1)                   # [1200, 400]
        # partition-major: [KT, NK*C], col = k*C + c
        xts[beta] = np.ascontiguousarray(
            XT.reshape(NK, KT, C).transpose(1, 0, 2).reshape(KT, NK * C))
    for g in range(PJ):
        gsl = slice(g * JL, (g + 1) * JL)
        # partition-major, k-major: col = k*JL + j
        wcs[g] = np.ascontiguousarray(
            Wneg[:, gsl].reshape(NK, KT, JL).transpose(1, 0, 2).reshape(
                KT, NK * JL))
        # coef [JS, 108]: col = kind*36 + jt*12 + la
        kinds = [M[:, gsl].reshape(NL, NJ, JS).transpose(2, 1, 0)
                 for M in (P, Q, R)]                            # [100, 3, 12]
        coefs[g] = np.ascontiguousarray(
            np.stack(kinds, axis=1).reshape(JS, 3 * NJ * NL).astype(bf16))

    in_maps = []
    for core in range(PB * PJ):
        beta, g = divmod(core, PJ)
        in_maps.append({"xt": xts[beta], "wc": wcs[g], "coef": coefs[g]})

    nc = _get_nc()
    res = run_bass_kernel_spmd(nc, in_maps, core_ids=list(range(PB * PJ)))
    kernel.last_results = res

    full = np.empty((B, NL, N), dtype=np.float32)
    for core in range(PB * PJ):
        beta, g = divmod(core, PJ)
        o = res.results[core]["out"]            # [JS, NJ*CA] bf16
        o = np.asarray(o).astype(np.float32).reshape(JS, NJ, NL, BL)
        # out[p, jt, la, b] -> full[b, la, g*JL + jt*JS + p]
        full[beta * BL:(beta + 1) * BL, :, g * JL:(g + 1) * JL] = (
            o.transpose(3, 2, 1, 0).reshape(BL, NL, JL))
    return full.reshape(B, NL, N, 1)
